# revision 65
# baseline (speedup 1.0000x reference)
"""Trainium2 Bass kernel for a ViT-style transformer block.

Reference computation (per batch element):
    h   = LN1(x);  qkv = h @ qkv_w.T + [q_bias, 0, v_bias]
    q,k,v per head (16 heads x 64);  attn = softmax(q*scale @ k.T + rel_bias)
    o   = (attn @ v) @ proj_w.T + proj_b;  x2 = x + o
    out = x2 + fc2(gelu(fc1(LN2(x2))))

Strategy: data-parallel over batch across 8 NeuronCores (8 samples each).
Weights (identical on every core) are folded/pre-transposed on the host at
first call and embedded in the NEFF as Const DRAM tensors, so the only
runtime input is x (bf16) and the only output is out (bf16).  Per core,
activations flow token-major through LN/residual (bn_stats + per-partition
tensor_scalar) and feature-major through the matmul chain.  All layout
switches (post-LN, post-attention) are PE-array transposes (identity
matmul into PSUM + ScalarE copy out) - no DMA transposes.  Attention
computes scores transposed ([keys, queries]) so exp needs no
cross-partition max (scores ~ N(0,1)); the attn@v matmul appends a ones
column to V so softmax denominators fall out of the same matmul.  proj_b
and fc2_b are accumulated into the same PSUM groups as their matmuls via
rank-1 ones-row matmuls.
"""

import sys
import zlib

sys.path.insert(0, "/opt/trn_rl_repo")

import numpy as np
import ml_dtypes

import concourse.bass as bass
import concourse.tile as tile
from concourse import mybir
from concourse.vector_clock import ScopedClock
from concourse.bass_utils import run_bass_kernel_spmd

F32 = mybir.dt.float32
BF16 = mybir.dt.bfloat16
AF = mybir.ActivationFunctionType
ALU = mybir.AluOpType

# Problem constants (hardcoded per spec)
B, N_TOK, D = 64, 197, 1024
NCORES = 8
BL = B // NCORES            # samples per core = 8
T = BL * N_TOK              # tokens per core = 1576
NH, HD = 16, 64             # heads
HID = 4096                  # MLP hidden
SCALE = HD ** -0.5
WH = WW = 14
NUM_REL = (2 * WH - 1) * (2 * WW - 1) + 3
LN_EPS = 1e-5

NT = (T + 127) // 128       # 13 token tiles
LASTP = T - 128 * (NT - 1)  # 40
KTP = [128, N_TOK - 128]    # per-sample key tile sizes [128, 69]
# token chunks for N-dim of feature-major matmuls
CHUNKS = [(i * 512, min(512, T - i * 512)) for i in range((T + 511) // 512)]
# fc2 token-tile groups (PSUM-bank limited)
FC2_GROUPS = [(0, 5), (5, 4), (9, 4)]


def _tok_tiles():
    """13 token tiles: (start, rows)."""
    return [(t * 128, 128 if t < NT - 1 else LASTP) for t in range(NT)]


def _sample_tiles():
    """16 sample-aligned key tiles: (b, kt, start_token, rows)."""
    out = []
    for b in range(BL):
        for kt in range(2):
            out.append((b, kt, b * N_TOK + kt * 128, KTP[kt]))
    return out


def _make_rel_pos_index():
    coords = np.stack(np.meshgrid(np.arange(WH), np.arange(WW), indexing="ij"))
    flat = coords.reshape(2, -1)
    rel = flat[:, :, None] - flat[:, None, :]
    rel = rel.transpose(1, 2, 0).copy()
    rel[:, :, 0] += WH - 1
    rel[:, :, 1] += WW - 1
    rel[:, :, 0] *= 2 * WW - 1
    idx = np.zeros((N_TOK, N_TOK), dtype=np.int32)
    idx[1:, 1:] = rel.sum(-1)
    idx[0, 0:] = NUM_REL - 3
    idx[0:, 0] = NUM_REL - 2
    idx[0, 0] = NUM_REL - 1
    return idx


class SplitDrainTileContext(tile.TileContext):
    """Walrus in this toolchain rejects >1 sync-wait on the kernel-tail
    Drain; split the waits across a chain of drain instructions."""

    def _drain_and_barrier(self, tick_clock, wait_clock):
        drain_inst = self.nc.sync.drain()
        wait_clock.add_sem_waits(
            drain_inst.ins, ScopedClock({None: tick_clock.global_clock})
        )
        si = drain_inst.ins.sync_info
        waits = list(si.on_wait) if si and si.on_wait else []
        if len(waits) > 1:
            si.on_wait = waits[:1]
            for w in waits[1:]:
                d2 = self.nc.sync.drain()
                si2 = d2.ins.sync_info
                if si2 is None:
                    d2.ins.sync_info = mybir.SyncInfo(on_wait=[w], on_update=[])
                else:
                    si2.on_wait = [w]
        self.nc.all_engine_barrier()
        assert self.sems is not None
        popped = self.nc._tile_sem_poison_stack.pop()
        assert popped is self._sem_poison
        self.nc.clear_and_free_semaphores(list(self.sems.allocated().values()))
        self.nc.all_engine_barrier()


def _layernorm_to_bf16(nc, pool, x_ap, p, eps_tile, out_bf16):
    """Token-major LN: x_ap [p,1024] -> out_bf16 [p,1024] bf16."""
    stats = pool.tile([128, 2, 6], F32, tag="ln_stats", name="ln_stats")
    for sg in range(2):
        nc.vector.bn_stats(out=stats[:p, sg, :], in_=x_ap[:, sg * 512:(sg + 1) * 512])
    mv = pool.tile([128, 2], F32, tag="ln_mv", name="ln_mv")
    nc.vector.bn_aggr(out=mv[:p, :], in_=stats[:p, :, :])
    rstd = pool.tile([128, 1], F32, tag="ln_rstd", name="ln_rstd")
    nc.scalar.activation(out=rstd[:p, :], in_=mv[:p, 1:2], func=AF.Sqrt,
                         bias=eps_tile[:p, :], scale=1.0)
    nc.vector.reciprocal(out=rstd[:p, :], in_=rstd[:p, :])
    nc.vector.tensor_scalar(
        out=out_bf16[:p, :], in0=x_ap, scalar1=mv[:p, 0:1], scalar2=rstd[:p, :],
        op0=ALU.subtract, op1=ALU.mult)


def _split_sync_waits(nc, cap=1):
    """Walrus in this toolchain caps sync-wait commands per instruction; hoist
    excess waits onto standalone event-semaphore instructions inserted just
    before the over-limit instruction on the same engine."""
    n = 0
    for fn in nc.m.functions:
        for bb in fn.blocks:
            insts = bb.instructions
            i = 0
            while i < len(insts):
                inst = insts[i]
                si = inst.sync_info
                waits = list(si.on_wait) if si and si.on_wait else []
                if len(waits) > cap and inst.engine != mybir.EngineType.Unassigned:
                    excess = waits[:len(waits) - cap]
                    si.on_wait = waits[len(waits) - cap:]
                    for w in excess:
                        ev = mybir.InstEventSemaphore(
                            name=f"waitsplit_{n}", ins=[], outs=[],
                            sync_info=mybir.SyncInfo(on_wait=[w], on_update=[]))
                        ev.engine = inst.engine
                        nc.register_instruction(ev)
                        insts.insert(i, ev)
                        n += 1
                        i += 1
                i += 1
    return n


def build_program(w):
    """w: host-prepared weight arrays (see prepare_weights)."""
    nc = bass.Bass("TRN2", target_bir_lowering=False, debug=False,
                   num_devices=NCORES, enable_partition_id=False)

    # ---- runtime I/O: x in, out out (both bf16) ----
    x_h = nc.declare_dram_parameter("x", [T, D], BF16, isOutput=False)
    out_h = nc.declare_dram_parameter("out", [T, D], BF16, isOutput=True)
    x2s_h = nc.dram_tensor("x2s", [T, D], F32)  # x2 scratch (residual base)

    # ---- weights baked into the NEFF (loaded to HBM at model-load) ----
    qkvwT_h = nc.inline_tensor(w["qkvwT"], name="c_qkvwT")
    qb_h = nc.inline_tensor(w["qb"], name="c_qb")
    vbrep_h = nc.inline_tensor(w["vb_rep"], name="c_vbrep")
    biasT_h = nc.inline_tensor(w["biasT"], name="c_biasT")
    projwT_h = nc.inline_tensor(w["projwT"], name="c_projwT")
    projb_h = nc.inline_tensor(w["projb_row"], name="c_projb")
    fc1wT_h = nc.inline_tensor(w["fc1wT"], name="c_fc1wT")
    fc1b_h = nc.inline_tensor(w["fc1b"], name="c_fc1b")
    fc2wT_h = nc.inline_tensor(w["fc2wT"], name="c_fc2wT")
    fc2b_h = nc.inline_tensor(w["fc2b_row"], name="c_fc2b")
    ident_h = nc.inline_tensor(
        np.eye(128, dtype=ml_dtypes.bfloat16), name="c_ident")

    tok_tiles = _tok_tiles()
    samp_tiles = _sample_tiles()

    with SplitDrainTileContext(nc) as tc:
        # Pool stack (LIFO release order):
        #   consts -> mid(ctxT,h2T) -> actA(qkt,vt) -> h1Tp -> [phase-scoped]
        consts_cm = tc.tile_pool(name="consts", bufs=1)
        consts = consts_cm.__enter__()
        eps_t = consts.tile([128, 1], F32, tag="eps", name="eps")
        nc.vector.memset(eps_t, LN_EPS)
        qb_t = consts.tile([128, 16], F32, tag="qb", name="qb")
        nc.sync.dma_start(out=qb_t, in_=qb_h[:, :])
        vbrep_t = consts.tile([128, D], F32, tag="vbrep", name="vbrep")
        nc.sync.dma_start(out=vbrep_t, in_=vbrep_h[:, :])
        ident_t = consts.tile([128, 128], BF16, tag="ident", name="ident")
        nc.sync.dma_start(out=ident_t, in_=ident_h[:, :])
        ones_t = consts.tile([1, 128], BF16, tag="ones", name="ones")
        nc.vector.memset(ones_t, 1.0)
        projb_t = consts.tile([1, D], BF16, tag="projb", name="projb")
        nc.sync.dma_start(out=projb_t, in_=projb_h[:, :])
        fc2b_t = consts.tile([1, D], BF16, tag="fc2b", name="fc2b")
        nc.sync.dma_start(out=fc2b_t, in_=fc2b_h[:, :])

        mid_cm = tc.tile_pool(name="mid", bufs=1)
        midp = mid_cm.__enter__()
        ctxT = [midp.tile([128, T], BF16, tag=f"ctxT{ft}", name=f"ctxT{ft}")
                for ft in range(8)]

        actA_cm = tc.tile_pool(name="actA", bufs=1)
        actA = actA_cm.__enter__()
        qkt = [actA.tile([128, T], BF16, tag=f"qkt{ft}", name=f"qkt{ft}")
               for ft in range(16)]
        vt = {}
        for (b, kt, t0, kp) in samp_tiles:
            vt[(b, kt)] = actA.tile([128, NH, 65], BF16, tag=f"v{b}_{kt}",
                                    name=f"v{b}_{kt}")

        h1T_cm = tc.tile_pool(name="h1Tp", bufs=1)
        h1Tp = h1T_cm.__enter__()
        h1T = h1Tp.tile([128, 8, T], BF16, tag="h1T", name="h1T")

        # qkv weights: Q/K tiles persist through the attention pipeline;
        # V tiles live in a sub-pool released right after the V matmuls
        qkvw_cm = tc.tile_pool(name="qkvw", bufs=1)
        qkvwp = qkvw_cm.__enter__()
        qw = [qkvwp.tile([128, D], BF16, tag=f"qkvw{i}", name=f"qkvw{i}")
              for i in range(16)]
        vw_cm = tc.tile_pool(name="vwp", bufs=1)
        vwp = vw_cm.__enter__()
        vw = [vwp.tile([128, D], BF16, tag=f"vw{i}", name=f"vw{i}")
              for i in range(8)]

        # ---------- Phase A: LN1 + PE transpose ----------
        with tc.tile_pool(name="xin", bufs=5) as xinp, \
             tc.tile_pool(name="ln1", bufs=3) as ln1p, \
             tc.tile_pool(name="tpA_ps", bufs=4, space="PSUM") as tpA:
            # first x tiles lead the DMA queue; V-weight prefetch follows
            xts = {}
            for (t0, p) in tok_tiles[:5]:
                xt = xinp.tile([128, D], BF16, tag="x_in", name="x_in")
                nc.sync.dma_start(out=xt[:p, :], in_=x_h[t0:t0 + p, :])
                xts[t0] = xt
            for kt in range(8):
                nc.sync.dma_start(out=vw[kt],
                                  in_=qkvwT_h[kt * 128:(kt + 1) * 128, 2 * D:3 * D])
            for (t0, p) in tok_tiles:
                if t0 in xts:
                    xt = xts[t0]
                else:
                    xt = xinp.tile([128, D], BF16, tag="x_in", name="x_in")
                    nc.sync.dma_start(out=xt[:p, :], in_=x_h[t0:t0 + p, :])
                h1 = ln1p.tile([128, D], BF16, tag="h1", name="h1")
                _layernorm_to_bf16(nc, ln1p, xt[:p, :], p, eps_t, h1)
                for kt in range(8):
                    pst = tpA.tile([128, 128], BF16, tag="tpA", name="tpA")
                    nc.tensor.transpose(
                        pst[:, :p], h1[:p, kt * 128:(kt + 1) * 128],
                        ident_t[:p, :p])
                    nc.scalar.copy(out=h1T[:, kt, t0:t0 + p], in_=pst[:, :p])
            # Q/K weight loads queue behind the LN-phase DMAs and stream
            # while the V matmuls below occupy the PE
            for half in range(2):
                for kt in range(8):
                    nc.sync.dma_start(
                        out=qw[half * 8 + kt],
                        in_=qkvwT_h[kt * 128:(kt + 1) * 128,
                                    half * D:(half + 1) * D])

            # ---------- Phase B1: V matmuls (token-major, sample-aligned) ----
            vps_cm = tc.tile_pool(name="v_ps", bufs=4, space="PSUM")
            vps_pool = vps_cm.__enter__()
            for (b, kt, t0, kp) in samp_tiles:
                vtile = vt[(b, kt)]
                nc.vector.memset(vtile[:, :, 64:65], 1.0)
                for vc in range(2):
                    ps = vps_pool.tile([128, 512], F32, tag="vps", name="vps")
                    for dk in range(8):
                        nc.tensor.matmul(
                            ps[:kp, :],
                            lhsT=h1T[:, dk, t0:t0 + kp],
                            rhs=vw[dk][:, vc * 512:(vc + 1) * 512],
                            start=(dk == 0), stop=(dk == 7))
                    nc.vector.tensor_add(
                        out=vtile[:kp, vc * 8:(vc + 1) * 8, 0:64],
                        in0=ps[:kp, :].rearrange("p (a d) -> p a d", a=8),
                        in1=vbrep_t[:kp, vc * 512:(vc + 1) * 512].rearrange(
                            "p (a d) -> p a d", a=8))
            vps_cm.__exit__(None, None, None)

        vw_cm.__exit__(None, None, None)  # V weights dead after the V matmuls

        # ---------- Phase B2+C: per-sample QK -> scores -> ctx pipeline ----
        # Each sample's Q/K columns (197 wide) are produced just before its
        # scores, so the PE stream interleaves QK matmuls with the
        # DVE/ACT-heavy softmax of the previous sample instead of idling.
        attp_cm = tc.tile_pool(name="attn_sb", bufs=1)
        attp = attp_cm.__enter__()
        bT = []
        for kt in range(2):
            t_ = attp.tile([128, NH, N_TOK], BF16, tag=f"biasT{kt}",
                           name=f"biasT{kt}")
            kp = KTP[kt]
            nc.sync.dma_start(out=t_[:kp, :, :],
                              in_=biasT_h[kt * 128: kt * 128 + kp, :, :])
            bT.append(t_)

        with tc.tile_pool(name="p_pool", bufs=2) as ppool, \
             tc.tile_pool(name="ctx_sb", bufs=3) as ctxp, \
             tc.tile_pool(name="qk_ps", bufs=2, space="PSUM") as qkps, \
             tc.tile_pool(name="sc_ps", bufs=1, space="PSUM") as scps, \
             tc.tile_pool(name="ctx_ps", bufs=1, space="PSUM") as ctxps, \
             tc.tile_pool(name="tpC_ps", bufs=2, space="PSUM") as tpC:

            pt = {}
            for b in range(BL):
                q0 = b * N_TOK
                # QK feature-major for this sample's 197 columns
                for half in range(2):
                    for fi in range(8):
                        ft = half * 8 + fi
                        ps = qkps.tile([128, 256], F32, tag="qkps", name="qkps")
                        for kt in range(8):
                            nc.tensor.matmul(
                                ps[:, :N_TOK],
                                lhsT=qw[half * 8 + kt][:, fi * 128:(fi + 1) * 128],
                                rhs=h1T[:, kt, q0:q0 + N_TOK],
                                start=(kt == 0), stop=(kt == 7))
                        nc.scalar.activation(
                            out=qkt[ft][:, q0:q0 + N_TOK], in_=ps[:, :N_TOK],
                            func=AF.Identity, bias=qb_t[:, ft:ft + 1], scale=1.0)

                # scores^T + exp, per key-tile, 2 heads per 2-bank PSUM group
                for kt in range(2):
                    kp = KTP[kt]
                    k0 = q0 + kt * 128
                    ptile = ppool.tile([128, NH, N_TOK], BF16, tag="P", name="P")
                    pt[(b, kt)] = ptile
                    for g in range(8):
                        ps = scps.tile([128, 2, 512], F32, tag="scps", name="scps")
                        for gi in range(2):
                            h = g * 2 + gi
                            ft = h // 2
                            rb = (h % 2) * 64
                            nc.tensor.matmul(
                                ps[:kp, gi, 0:N_TOK],
                                lhsT=qkt[8 + ft][rb:rb + 64, k0:k0 + kp],
                                rhs=qkt[ft][rb:rb + 64, q0:q0 + N_TOK],
                                start=True, stop=True)
                        psl = ptile[:kp, g * 2:(g + 1) * 2, :]
                        nc.vector.tensor_add(
                            out=psl,
                            in0=ps[:kp, :, 0:N_TOK],
                            in1=bT[kt][:kp, g * 2:(g + 1) * 2, :])
                        nc.scalar.activation(out=psl, in_=psl, func=AF.Exp)

                # ctx token-major with fused sumexp (ones column of V);
                # 8 heads per pass
                for qt in range(2):
                    qn = KTP[qt]
                    qoff = qt * 128
                    c0 = b * N_TOK + qoff
                    for hh in range(2):
                        ps = ctxps.tile([128, 8, 128], F32, tag="ctxps",
                                        name="ctxps")
                        for hi in range(8):
                            h = hh * 8 + hi
                            for kt in range(2):
                                kp = KTP[kt]
                                nc.tensor.matmul(
                                    ps[:qn, hi, 0:65],
                                    lhsT=pt[(b, kt)][:kp, h, qoff:qoff + qn],
                                    rhs=vt[(b, kt)][:kp, h, :],
                                    start=(kt == 0), stop=(kt == 1))
                        rec = ctxp.tile([128, 8], F32, tag="rec", name="rec")
                        nc.vector.reciprocal(out=rec[:qn, :], in_=ps[:qn, :, 64])
                        cs = ctxp.tile([128, 8, HD], BF16, tag="ctx", name="ctx")
                        nc.vector.tensor_mul(
                            out=cs[:qn, :, :],
                            in0=ps[:qn, :, 0:64],
                            in1=rec[:qn, :, None].broadcast_to([qn, 8, HD]))
                        # PE transpose to feature-major ctxT
                        for blk in range(4):
                            pst = tpC.tile([128, 128], BF16, tag="tpC",
                                           name="tpC")
                            nc.tensor.transpose(
                                pst[:, :qn], cs[:qn, blk * 2:blk * 2 + 2, :],
                                ident_t[:qn, :qn])
                            nc.scalar.copy(
                                out=ctxT[hh * 4 + blk][:, c0:c0 + qn],
                                in_=pst[:, :qn])

        attp_cm.__exit__(None, None, None)
        qkvw_cm.__exit__(None, None, None)
        h1T_cm.__exit__(None, None, None)
        actA_cm.__exit__(None, None, None)

        h2T_cm = tc.tile_pool(name="h2Tp", bufs=1)
        h2Tp = h2T_cm.__enter__()
        h2T = h2Tp.tile([128, 8, T], BF16, tag="h2T", name="h2T")

        lateC_cm = tc.tile_pool(name="lateC", bufs=1)
        lateC = lateC_cm.__enter__()
        fc1b_t = lateC.tile([128, 32], F32, tag="fc1b", name="fc1b")
        nc.sync.dma_start(out=fc1b_t, in_=fc1b_h[:, :])

        # ---------- Phase D: proj + residual + LN2 ----------
        with tc.tile_pool(name="projw", bufs=1) as projwp, \
             tc.tile_pool(name="proj_ps", bufs=2, space="PSUM") as projps, \
             tc.tile_pool(name="proj_sb", bufs=3) as projsb, \
             tc.tile_pool(name="tpD_ps", bufs=6, space="PSUM") as tpD:
            pw = [projwp.tile([128, D], BF16, tag=f"projw{kt}", name=f"projw{kt}")
                  for kt in range(8)]
            for kt in range(8):
                nc.sync.dma_start(out=pw[kt], in_=projwT_h[kt * 128:(kt + 1) * 128, :])
            for (t0, p) in tok_tiles:
                xr = projsb.tile([128, D], BF16, tag="xres", name="xres")
                nc.sync.dma_start(out=xr[:p, :], in_=x_h[t0:t0 + p, :])
                x2 = projsb.tile([128, D], F32, tag="x2", name="x2")
                for f in range(2):
                    ps = projps.tile([128, 512], F32, tag="projps", name="projps")
                    for kt in range(8):
                        nc.tensor.matmul(
                            ps[:p, :],
                            lhsT=ctxT[kt][:, t0:t0 + p],
                            rhs=pw[kt][:, f * 512:(f + 1) * 512],
                            start=(kt == 0), stop=False)
                    nc.tensor.matmul(
                        ps[:p, :],
                        lhsT=ones_t[0:1, :p],
                        rhs=projb_t[0:1, f * 512:(f + 1) * 512],
                        start=False, stop=True)
                    nc.vector.tensor_add(
                        out=x2[:p, f * 512:(f + 1) * 512],
                        in0=ps[:p, :], in1=xr[:p, f * 512:(f + 1) * 512])
                # x2 -> HBM scratch (residual base for fc2 drain)
                nc.sync.dma_start(out=x2s_h[t0:t0 + p, :], in_=x2[:p, :])
                # LN2 -> h2 bf16 -> PE transpose
                h2 = projsb.tile([128, D], BF16, tag="h2", name="h2")
                _layernorm_to_bf16(nc, projsb, x2[:p, :], p, eps_t, h2)
                for kt in range(8):
                    pst = tpD.tile([128, 128], BF16, tag="tpD", name="tpD")
                    nc.tensor.transpose(
                        pst[:, :p], h2[:p, kt * 128:(kt + 1) * 128],
                        ident_t[:p, :p])
                    nc.scalar.copy(out=h2T[:, kt, t0:t0 + p], in_=pst[:, :p])

        # ---------- Phase E: MLP ----------
        gT_cm = tc.tile_pool(name="gT_pool", bufs=1)
        gTp = gT_cm.__enter__()
        gT = gTp.tile([128, 32, T], BF16, tag="gT", name="gT")
        with tc.tile_pool(name="fc1w", bufs=5) as fc1wp, \
             tc.tile_pool(name="fc1_ps", bufs=2, space="PSUM") as fc1ps:
            wts = []
            for Ht in range(32):
                wt = fc1wp.tile([128, D], BF16, tag="fc1w", name="fc1w")
                nc.sync.dma_start(out=wt, in_=fc1wT_h[Ht, :, :])
                wts.append(wt)
            for Ht in range(32):
                wt = wts[Ht]
                for (c0, cw) in CHUNKS:
                    ps = fc1ps.tile([128, 512], F32, tag="fc1ps", name="fc1ps")
                    for kt in range(8):
                        nc.tensor.matmul(
                            ps[:, :cw],
                            lhsT=wt[:, kt * 128:(kt + 1) * 128],
                            rhs=h2T[:, kt, c0:c0 + cw],
                            start=(kt == 0), stop=(kt == 7))
                    nc.scalar.activation(
                        out=gT[:, Ht, c0:c0 + cw], in_=ps[:, :cw],
                        func=AF.Gelu, bias=fc1b_t[:, Ht:Ht + 1], scale=1.0)

        # fc2: f-inner (each [128,1024] weight tile DMA'd once per group and
        # its gT lhsT slice reused for both output halves); weight + residual
        # tiles prefetched at group start.
        with tc.tile_pool(name="fc2w", bufs=16) as fc2wp, \
             tc.tile_pool(name="fc2_ps", bufs=8, space="PSUM") as fc2ps, \
             tc.tile_pool(name="fc2_xr", bufs=4) as fc2xr, \
             tc.tile_pool(name="fc2_sb", bufs=4) as fc2sb:
            # the degenerate 1-tile group sits between full groups so its
            # weight re-stream hides under the neighbours' compute
            for (g0, gn) in ((0, 4), (4, 4), (8, 4), (12, 1)):
                pss = [[fc2ps.tile([128, 512], F32, tag="fc2ps", name="fc2ps")
                        for _ in range(2)] for _ in range(gn)]
                xfs = [[None, None] for _ in range(gn)]
                for i in range(gn):
                    t0, p = tok_tiles[g0 + i]
                    for f in range(2):
                        xf = fc2xr.tile([128, 512], F32, tag="x2_in",
                                        name="x2_in")
                        nc.sync.dma_start(
                            out=xf[:p, :],
                            in_=x2s_h[t0:t0 + p, f * 512:(f + 1) * 512])
                        xfs[i][f] = xf
                w2s = []
                for Hkt in range(32):
                    w2 = fc2wp.tile([128, D], BF16, tag="fc2w", name="fc2w")
                    nc.sync.dma_start(
                        out=w2, in_=fc2wT_h[Hkt * 128:(Hkt + 1) * 128, :])
                    w2s.append(w2)
                for Hkt in range(32):
                    for i in range(gn):
                        t0, p = tok_tiles[g0 + i]
                        for f in range(2):
                            nc.tensor.matmul(
                                pss[i][f][:p, :],
                                lhsT=gT[:, Hkt, t0:t0 + p],
                                rhs=w2s[Hkt][:, f * 512:(f + 1) * 512],
                                start=(Hkt == 0), stop=False)
                for i in range(gn):
                    t0, p = tok_tiles[g0 + i]
                    for f in range(2):
                        nc.tensor.matmul(
                            pss[i][f][:p, :],
                            lhsT=ones_t[0:1, :p],
                            rhs=fc2b_t[0:1, f * 512:(f + 1) * 512],
                            start=False, stop=True)
                for i in range(gn):
                    t0, p = tok_tiles[g0 + i]
                    for f in range(2):
                        ot = fc2sb.tile([128, 512], BF16, tag="out_sb",
                                        name="out_sb")
                        nc.vector.tensor_add(out=ot[:p, :], in0=pss[i][f][:p, :],
                                             in1=xfs[i][f][:p, :])
                        nc.sync.dma_start(
                            out=out_h[t0:t0 + p, f * 512:(f + 1) * 512],
                            in_=ot[:p, :])
        gT_cm.__exit__(None, None, None)
        lateC_cm.__exit__(None, None, None)
        h2T_cm.__exit__(None, None, None)
        mid_cm.__exit__(None, None, None)
        consts_cm.__exit__(None, None, None)
    _split_sync_waits(nc)
    return nc


def prepare_weights(qkv_w, q_bias, v_bias, rel_bias_table, proj_w, proj_b,
                    ln1_g, ln1_b, ln2_g, ln2_b, fc1_w, fc1_b, fc2_w, fc2_b):
    """Fold LN affine params / scale into weights; pre-transpose; gather
    rel-pos bias.  Returns the dict build_program() embeds in the NEFF."""
    bf = ml_dtypes.bfloat16
    f32 = np.float32

    # fold LN1 gamma/beta into qkv weights, scale q by 1/8
    qkv_b = np.concatenate([q_bias, np.zeros_like(v_bias), v_bias]).astype(f32)
    W1 = qkv_w.astype(f32) * ln1_g[None, :].astype(f32)
    b1 = qkv_b + qkv_w.astype(f32) @ ln1_b.astype(f32)
    W1[:D] *= SCALE
    b1[:D] *= SCALE
    qkvwT = np.ascontiguousarray(W1.T).astype(bf)          # [1024, 3072]
    qb = np.ascontiguousarray(b1[:2 * D].reshape(16, 128).T).astype(f32)  # [128,16]
    vb_rep = np.broadcast_to(b1[2 * D:], (128, D)).copy().astype(f32)

    # rel-pos bias, transposed to [k, h, q]
    idx = _make_rel_pos_index()
    rel = rel_bias_table.astype(f32)[idx]                  # [q, k, h]
    biasT = np.ascontiguousarray(rel.transpose(1, 2, 0)).astype(bf)  # [k, h, q]

    projwT = np.ascontiguousarray(proj_w.astype(f32).T).astype(bf)    # [1024,1024]
    projb_row = np.ascontiguousarray(proj_b.astype(f32)[None, :]).astype(bf)

    # fold LN2 gamma/beta into fc1
    W3 = fc1_w.astype(f32) * ln2_g[None, :].astype(f32)
    b3 = fc1_b.astype(f32) + fc1_w.astype(f32) @ ln2_b.astype(f32)
    W3T = np.ascontiguousarray(W3.T)                       # [1024, 4096]
    fc1wT = W3T.reshape(8, 128, 32, 128).transpose(2, 1, 0, 3)
    fc1wT = np.ascontiguousarray(fc1wT.reshape(32, 128, D)).astype(bf)
    fc1b = np.ascontiguousarray(b3.reshape(32, 128).T).astype(f32)    # [128,32]

    fc2wT = np.ascontiguousarray(fc2_w.astype(f32).T).astype(bf)      # [4096,1024]
    fc2b_row = np.ascontiguousarray(fc2_b.astype(f32)[None, :]).astype(bf)

    return dict(qkvwT=qkvwT, qb=qb, vb_rep=vb_rep, biasT=biasT,
                projwT=projwT, projb_row=projb_row, fc1wT=fc1wT, fc1b=fc1b,
                fc2wT=fc2wT, fc2b_row=fc2b_row)


def prepare_host_inputs(x, **_weights):
    """Per-core runtime input maps: just x, cast to bf16."""
    xbf = np.asarray(x, np.float32).astype(ml_dtypes.bfloat16)
    in_maps = []
    for c in range(NCORES):
        sl = slice(c * BL, (c + 1) * BL)
        in_maps.append(
            {"x": np.ascontiguousarray(xbf[sl].reshape(T, D))})
    return in_maps


_CACHED = {"fp": None, "nc": None}


def _weights_fingerprint(w):
    h = 0
    for k in sorted(w):
        a = np.ascontiguousarray(w[k])
        h = zlib.adler32(a.tobytes(), h)
        h = zlib.adler32(str(a.shape).encode(), h)
    return h


def _get_nc():
    assert _CACHED["nc"] is not None, "kernel() must run once to build"
    return _CACHED["nc"]


def kernel(**inputs):
    # Fingerprint the raw weight inputs (full bytes) so warm calls skip the
    # host-side weight folding entirely; prepared weights are a pure
    # function of the raw ones, so this is equivalent to hashing them.
    fp = _weights_fingerprint(
        {k: v for k, v in inputs.items() if k != "x"})
    if _CACHED["fp"] != fp:
        w = prepare_weights(**{k: v for k, v in inputs.items() if k != "x"})
        _CACHED["nc"] = build_program(w)
        _CACHED["fp"] = fp
    nc = _CACHED["nc"]
    in_maps = prepare_host_inputs(**inputs)
    res = run_bass_kernel_spmd(nc, in_maps, list(range(NCORES)))
    outs = [res.results[c]["out"].astype(np.float32).reshape(BL, N_TOK, D)
            for c in range(NCORES)]
    return np.concatenate(outs, axis=0)
